# revision 41
# baseline (speedup 1.0000x reference)
"""Trainium2 Bass kernel for nn_LocalAggregator (GNN message passing).

Math (per batch):
    e[i,j,r] = lrelu( h_i . diag(a_r) . h_j  +  g_r(A_ij) )
    with g_r(a) = sum_t cos(a f_t + p_t) iw[t,r]
    s[i,j]   = e[i,j,adj_ij-1]  if 1<=adj<=5 else -9e15
    out      = softmax_j(s) @ h

Device strategy (per core, BL=4 of the 32 batches). Everything is computed
TRANSPOSED — scores live as s^T[j, (b,i)] — which removes the PE transpose
and yields the softmax denominator from an appended ones-column:
  * e1_c = H diag(a_c) H^T is symmetric, so psum[j, (c,i)-blocks] are fed
    by stationary hT_b (raw fp16) x moving hTa (a_c-scaled hT fp16, classes
    grouped in the moving free dim): per (b,ch) one 384-col + one 256-col
    fp16 matmul (1 cycle/row; plain fp32 is 4). One accumulation group per
    PSUM bank — interleaved open groups sharing a bank drop partial sums.
  * g_r(A): host fits a degree-4 polynomial per class AND RESOLVES THE
    CLASS SELECTION AT PACK TIME into per-element coefficient planes
    C_k[j,(b,i)] = c_k^(adj), shipped as int16 with a per-degree scale
    that folds into the STT for free:  t = (I_k * s_k) + t ; t = t * A.
    7 DVE ops total, no per-class work. The constant term c_0^(c) is
    accumulated into the e1 psum group by 1-row ones-stationary matmuls
    (emitted first, as the group start, in the PE-idle warmup window).
  * Class select for e1: Act broadcasts NEG_INF into s, then 5 DVE
    copy_predicated per b-pair half (int8 masks, straight from PSUM);
    poly fold + leaky relu are 2 more DVE stts per half (masked elements
    stay ~-9e15 -> exp == 0).
  * Softmax: exp(s - 8) on Act into fp16 (bias cancels in alpha; logit max
    is ~15.7 for this model so e^{s-8} fits fp16); denominator from the
    ones-column of the final fp16 matmul  alpha^T @ [h | 1 | pad]  (260
    cols; fp32r needs multiples of 4); per-partition 1/Z applied during
    the PSUM->SBUF copy on Act; output shipped fp16.
  * hTa build split DVE (fp16 tensor_scalar hits the 4x packed mode,
    ~345ns/op) / Act; madj+h1 ride the gpsimd software-DGE queue as a
    third DMA channel (safe only for late-consumed tensors).
  * Two walrus version-skew workarounds: the Tile tail drain and any
    instruction may carry at most ONE sync-wait command on this toolchain
    (_patch_tail_drain / _split_excess_waits hoist excess waits onto
    NoOps). Custom DVE ops and gpsimd wide tensor ops don't work on this
    toolchain/HW (codegen "ISA wrong length" / ~7.6us per 512-col op).
"""

from contextlib import ExitStack

import numpy as np

B, N, D, TDIM = 32, 128, 256, 64
NCORES = 8
BL = B // NCORES            # batches per core
ALPHA = 0.2
NEG_INF = -9e15
DEG = 4                     # host-fitted polynomial degree (fit err ~2e-3)
DEBUG_DUMP = False
DCH = D // 128              # K-chunks for the e1 contraction

_PROG_CACHE: dict = {}
_DRAIN_PATCHED = False


def _patch_tail_drain():
    """Version-skew workaround: the TileContext tail drain accumulates one
    sem-wait per outstanding engine/DMA queue, but this walrus build's Drain
    encoding fits only ONE sync-wait command. Spread the excess waits over
    preceding single-wait NoOps on the same (SP) engine."""
    global _DRAIN_PATCHED
    if _DRAIN_PATCHED:
        return
    import concourse.tile as tile_mod

    def _patched(self, tick_clock, wait_clock):
        nc = self.nc
        drain_inst = nc.sync.drain()
        wait_clock.add_sem_waits(
            drain_inst.ins,
            tile_mod.ScopedClock({None: tick_clock.global_clock}),
        )
        mi = drain_inst.ins
        si = mi.sync_info
        waits = list(si.on_wait) if si is not None and si.on_wait else []
        if len(waits) > 1:
            si.on_wait = waits[:1]
            lst = nc.cur_bb.bb.instructions
            assert lst[-1] is mi, "drain is not the last instruction in block"
            drain_obj = lst.pop()
            for w in waits[1:]:
                nop = nc.sync.nop(nofuse=True)
                nsi = nop.ins.sync_info
                if nsi is None:
                    nop.ins.sync_info = type(si)(on_update=[], on_wait=[w])
                else:
                    nsi.on_wait = [w]
            lst.append(drain_obj)
        nc.all_engine_barrier()
        assert self.sems is not None
        popped = nc._tile_sem_poison_stack.pop()
        assert popped is self._sem_poison
        nc.clear_and_free_semaphores(list(self.sems.allocated().values()))
        nc.all_engine_barrier()

    tile_mod.TileContext._drain_and_barrier = _patched
    _DRAIN_PATCHED = True


def _split_excess_waits(nc, max_waits: int = 1):
    """This walrus build encodes at most one sync-wait command per
    instruction. Hoist excess waits onto same-engine NoOps inserted
    immediately before the over-subscribed instruction."""
    import concourse.mybir as mybir

    for fn in nc.m.functions:
        for bb in fn.blocks:
            insts = bb.instructions
            i = 0
            while i < len(insts):
                inst = insts[i]
                si = getattr(inst, "sync_info", None)
                waits = list(si.on_wait) if si is not None and si.on_wait else []
                if len(waits) > max_waits:
                    si.on_wait = waits[:max_waits]
                    extra = waits[max_waits:]
                    nops = []
                    for k in range(0, len(extra), max_waits):
                        nops.append(
                            mybir.InstNoOp(
                                name=f"{inst.name}-xw{k}",
                                engine=inst.engine,
                                bass_nofuse=True,
                                sync_info=mybir.SyncInfo(
                                    on_wait=extra[k : k + max_waits], on_update=[]
                                ),
                            )
                        )
                    insts[i:i] = nops
                    i += len(nops)
                i += 1


# --------------------------------------------------------------------------
# host-side parameter preprocessing
# --------------------------------------------------------------------------
def _fit_polys(iw_params: np.ndarray, te_freq: np.ndarray, te_phase: np.ndarray):
    """Least-squares fit of g_c(a) = sum_t iw[t,c] cos(a f_t + p_t), a in [0,1].

    Returns C[k, c] for k=0..DEG (monomial basis, increasing order).
    """
    npts = 1024
    x = 0.5 * (1.0 + np.cos(np.pi * (np.arange(npts) + 0.5) / npts))
    f = te_freq.astype(np.float64)
    p = te_phase.astype(np.float64)
    iw = iw_params.astype(np.float64)
    G = np.cos(x[:, None] * f[None, :] + p[None, :]) @ iw      # (npts, 5)
    V = np.vander(x, DEG + 1, increasing=True)                 # (npts, DEG+1)
    C, *_ = np.linalg.lstsq(V, G, rcond=None)
    return C  # (DEG+1, 5) float64


# --------------------------------------------------------------------------
# Bass program
# --------------------------------------------------------------------------
def _build_program(Cpoly: np.ndarray):
    """One-core program; SPMD across 8 cores with per-core input maps."""
    CSCALE = np.abs(Cpoly).max(axis=1) / 32767.0
    import concourse.bass as bass
    import concourse.mybir as mybir
    import concourse.tile as tile

    _patch_tail_drain()

    f32 = mybir.dt.float32
    f32r = mybir.dt.float32r
    f16 = mybir.dt.float16
    i8 = mybir.dt.int8
    Alu = mybir.AluOpType
    Act = mybir.ActivationFunctionType

    nc = bass.Bass()

    FBJ = BL * N          # 512  free size of (b, x)
    FBD = BL * D          # 1024 free size of (b, d)
    HTC = DCH * FBJ       # 1024 hT cols (ch, b, n)

    # DRAM I/O (per-core layouts; host arranges)
    hT_d = nc.dram_tensor("hT", [128, HTC], f16, kind="ExternalInput")
    i16 = mybir.dt.int16
    fbC1_d = nc.dram_tensor("fbC1", [N, FBJ], i16, kind="ExternalInput")     # C4
    fbC3_d = nc.dram_tensor("fbC3", [N, 3 * FBJ], i16, kind="ExternalInput")  # C3 C2 C1
    c0_d = nc.dram_tensor("c0row", [1, 128 + 640], f32r, kind="ExternalInput")
    madj_d = nc.dram_tensor("madj", [N, 5 * FBJ], i8, kind="ExternalInput")
    h1_d = nc.dram_tensor("h1", [N, BL * (D + 4)], f16, kind="ExternalInput")
    ap_d = nc.dram_tensor("ap", [128, DCH * 5 + 2], f32, kind="ExternalInput")
    A_d = nc.dram_tensor("A", [N, FBJ], f16, kind="ExternalInput")
    out_d = nc.dram_tensor("out", [N, FBD], f16, kind="ExternalOutput")  # [i,(b,d)]
    if DEBUG_DUMP:
        dbgA_d = nc.dram_tensor("dbgA", [N, BL * 512], f32, kind="ExternalOutput")
        dbgB_d = nc.dram_tensor("dbgB", [N, BL * 512], f32, kind="ExternalOutput")
        dbgT_d = nc.dram_tensor("dbgT", [N, FBJ], f32, kind="ExternalOutput")
        dbgS_d = nc.dram_tensor("dbgS", [N, FBJ], f32, kind="ExternalOutput")

    with tile.TileContext(nc) as tc, ExitStack() as ctx:
        io = ctx.enter_context(tc.tile_pool(name="io", bufs=1))
        wrk = ctx.enter_context(tc.tile_pool(name="wrk", bufs=1))

        # ---- loads: sync queue feeds Act/PE chain, scalar queue feeds DVE
        s_sb = wrk.tile([N, FBJ], f32, tag="s")
        hT_sb = io.tile([128, HTC], f16, tag="hT")
        nc.sync.dma_start(hT_sb[:, 0:FBJ], hT_d[:, 0:FBJ])
        c0_sb = io.tile([1, 768], f32r, tag="c0row")
        nc.sync.dma_start(c0_sb[:], c0_d[:])
        fbC1_sb = io.tile([N, FBJ], i16, tag="fbC1")
        nc.sync.dma_start(fbC1_sb[:], fbC1_d[:])
        nc.sync.dma_start(hT_sb[:, FBJ : 2 * FBJ], hT_d[:, FBJ : 2 * FBJ])
        ap_sb_t = io.tile([128, DCH * 5 + 2], f32, tag="ap")
        nc.scalar.dma_start(ap_sb_t[:], ap_d[:])
        ap_sb = ap_sb_t[:]
        A16_sb = io.tile([N, FBJ], f16, tag="A16")
        nc.scalar.dma_start(A16_sb[:], A_d[:])
        A_sb_t = wrk.tile([N, FBJ], f32, tag="A")
        nc.scalar.copy(A_sb_t[:], A16_sb[:])
        A_sb = A_sb_t[:]
        fbC3_sb = io.tile([N, 3 * FBJ], i16, tag="fbC3")
        nc.scalar.dma_start(fbC3_sb[:], fbC3_d[:])
        madj_sb = io.tile([N, 5 * FBJ], i8, tag="madj")
        nc.gpsimd.dma_start(madj_sb[:], madj_d[:])
        h1_sb = io.tile([N, BL * (D + 4)], f16, tag="h1")
        nc.gpsimd.dma_start(h1_sb[:], h1_d[:])

        # ---- hTa[dl, (ch, c, b, n)] = a_c[dl] * hT  (Act for c<3, Pool c>=3)
        hTa = wrk.tile([128, DCH * 5 * FBJ], f16, tag="hTa")
        for ch in range(DCH):
            for c in range(5):
                dst = hTa[:, (ch * 5 + c) * FBJ : (ch * 5 + c + 1) * FBJ]
                hsrc = hT_sb[:, ch * FBJ : (ch + 1) * FBJ]
                scal = ap_sb[:, ch * 5 + c : ch * 5 + c + 1]
                if ch == 1 and c >= 3:
                    nc.scalar.mul(dst, hsrc, scal)
                else:
                    nc.vector.tensor_scalar(dst, hsrc, scal, None, Alu.mult)
        hTa_v = hTa[:].rearrange("p (ch c b n) -> p ch c b n", ch=DCH, c=5, b=BL)

        # ---- DVE: t[j,(b,i)] = (((C4*A + C3)*A + C2)*A + C1)*A
        # (per-element class-selected coefficient planes; c0 comes via PE)
        t_sb = wrk.tile([N, FBJ], f32, tag="t")
        nc.vector.scalar_tensor_tensor(
            t_sb[:], fbC1_sb[:], float(CSCALE[4]), A_sb,
            Alu.mult, Alu.mult)
        for sk, cpl in [(float(CSCALE[3]), fbC3_sb[:, 0:FBJ]),
                        (float(CSCALE[2]), fbC3_sb[:, FBJ : 2 * FBJ]),
                        (float(CSCALE[1]), fbC3_sb[:, 2 * FBJ : 3 * FBJ])]:
            nc.vector.scalar_tensor_tensor(
                t_sb[:], cpl, sk, t_sb[:], Alu.mult, Alu.add)
            nc.vector.scalar_tensor_tensor(
                t_sb[:], t_sb[:], 1.0, A_sb, Alu.mult, Alu.mult)

        # ---- PE: e1 + c0 accumulation, fp32r
        # psA[j, b*512 + c*128 + i] c in 0..2 ; psB[j, b*256 + (c-3)*128 + i]
        psum_cm = tc.tile_pool(name="psum", bufs=1, space="PSUM")
        psum = psum_cm.__enter__()
        psA = psum.tile([N, BL * 512], f32, tag="psA")
        psB = psum.tile([N, BL * 512], f32, tag="psB")
        psA_v = psA[:].rearrange("p (b x) -> p b x", b=BL)
        psB_v = psB[:].rearrange("p (b x) -> p b x", b=BL)

        ones_st = c0_sb[:, 0:128]
        c0A = c0_sb[:, 128:512]          # (c,i) c in 0..2
        c0B = c0_sb[:, 512:768]          # (c,i) c in 3..4
        for b in range(BL):
            nc.tensor.matmul(psA[:, b * 512 : b * 512 + 384], ones_st, c0A,
                             start=True, stop=False)
            nc.tensor.matmul(psB[:, b * 512 : b * 512 + 256], ones_st, c0B,
                             start=True, stop=False)
        for b in range(BL):
            stat = hT_sb[:, b * N : (b + 1) * N]
            nc.tensor.matmul(psA[:, b * 512 : b * 512 + 384], stat,
                             hTa_v[:, 0, 0:3, b, :], start=False, stop=False)
            nc.tensor.matmul(psB[:, b * 512 : b * 512 + 256], stat,
                             hTa_v[:, 0, 3:5, b, :], start=False, stop=False)
        for b in range(BL):
            stat = hT_sb[:, FBJ + b * N : FBJ + (b + 1) * N]
            nc.tensor.matmul(psA[:, b * 512 : b * 512 + 384], stat,
                             hTa_v[:, 1, 0:3, b, :], start=False, stop=True)
            nc.tensor.matmul(psB[:, b * 512 : b * 512 + 256], stat,
                             hTa_v[:, 1, 3:5, b, :], start=False, stop=True)

        if DEBUG_DUMP:
            dbgA_sb = wrk.tile([N, BL * 512], f32, tag="dbgA")
            nc.vector.tensor_copy(dbgA_sb[:], psA[:])
            nc.sync.dma_start(dbgA_d[:], dbgA_sb[:])
            dbgB_sb = wrk.tile([N, BL * 512], f32, tag="dbgB")
            nc.vector.tensor_copy(dbgB_sb[:], psB[:])
            nc.sync.dma_start(dbgB_d[:], dbgB_sb[:])

        # ---- select: s = NEG_INF ; copy_predicated class planes from PSUM
        madj_v = madj_sb[:].rearrange("p (m b i) -> p m b i", m=5, b=BL)
        s_v = s_sb[:].rearrange("p (b i) -> p b i", b=BL)
        neg_bc = ap_sb[:, DCH * 5 : DCH * 5 + 1].broadcast_to((N, FBJ))
        nc.scalar.activation(s_sb[:], neg_bc, Act.Copy)
        for half in range(2):
            bh = slice(half * 2, half * 2 + 2)
            for r in range(5):
                src = (psA_v[:, bh, r * 128 : (r + 1) * 128] if r < 3
                       else psB_v[:, bh, (r - 3) * 128 : (r - 2) * 128])
                nc.vector.copy_predicated(s_v[:, bh], madj_v[:, r, bh], src)
        psum_cm.__exit__(None, None, None)

        if DEBUG_DUMP:
            nc.sync.dma_start(dbgT_d[:], t_sb[:])
            nc.sync.dma_start(dbgS_d[:], s_sb[:])

        # ---- Act: exp ; PE: alpha^T @ [h|1] ; Act: 1/Z scale ; DMA out
        ex = wrk.tile([N, FBJ], f16, tag="ex")
        rz = wrk.tile([N, BL], f32, tag="rz")
        out_sb = wrk.tile([N, FBD], f16, tag="out_sb")
        psum2 = ctx.enter_context(tc.tile_pool(name="psum2", bufs=2, space="PSUM"))
        for half in range(2):
            hs_ = slice(half * 256, (half + 1) * 256)
            # fold poly branch + leaky relu (per half, overlaps exp)
            nc.vector.scalar_tensor_tensor(
                s_sb[:, hs_], s_sb[:, hs_], 0.0, t_sb[:, hs_],
                Alu.add, Alu.add)
            nc.vector.scalar_tensor_tensor(
                s_sb[:, hs_], s_sb[:, hs_], ALPHA, s_sb[:, hs_],
                Alu.mult, Alu.max)
            for bb in (half * 2, half * 2 + 1):
                nc.scalar.activation(
                    ex[:, bb * N : (bb + 1) * N], s_sb[:, bb * N : (bb + 1) * N],
                    Act.Exp, bias=ap_sb[:, DCH * 5 + 1 : DCH * 5 + 2],
                )
        del half
        for b in range(BL):
            op2 = psum2.tile([N, D + 4], f32, tag="op2", name="op2")
            nc.tensor.matmul(
                op2[:],
                ex[:, b * N : (b + 1) * N],
                h1_sb[:, b * (D + 4) : (b + 1) * (D + 4)],
            )
            nc.vector.reciprocal(rz[:, b : b + 1], op2[:, D : D + 1])
            nc.scalar.mul(out_sb[:, b * D : (b + 1) * D], op2[:, 0:D],
                          rz[:, b : b + 1])
            nc.sync.dma_start(
                out_d[:, b * D : (b + 1) * D], out_sb[:, b * D : (b + 1) * D])

    return nc


# --------------------------------------------------------------------------
# host-side input prep (shared by kernel() and test.py's profiler)
# --------------------------------------------------------------------------
def _prepare(inputs):
    hidden = np.ascontiguousarray(inputs["hidden"], dtype=np.float32)   # (B,N,D)
    A = np.ascontiguousarray(inputs["A_interval"], dtype=np.float32)    # (B,N,N)
    adj = np.asarray(inputs["adj"])                                     # (B,N,N) i32
    a_params = np.asarray(inputs["a_params"], dtype=np.float32)         # (D,5)
    iw = np.asarray(inputs["iw_params"])
    f = np.asarray(inputs["te_freq"])
    p = np.asarray(inputs["te_phase"])

    Cpoly = _fit_polys(iw, f, p)

    key = Cpoly.tobytes()
    nc = _PROG_CACHE.get(key)
    if nc is None:
        nc = _build_program(Cpoly)
        _split_excess_waits(nc)
        _PROG_CACHE[key] = nc

    # a_params -> [dl, (ch, c)]
    ap_host = np.empty((128, DCH * 5 + 2), np.float32)
    for ch in range(DCH):
        ap_host[:, ch * 5 : (ch + 1) * 5] = a_params[ch * 128 : (ch + 1) * 128, :]
    ap_host[:, DCH * 5] = NEG_INF
    ap_host[:, DCH * 5 + 1] = -8.0
    c0_host = np.empty((1, 768), np.float32)
    c0_host[0, 0:128] = 1.0
    for c in range(5):
        c0_host[0, 128 + c * 128 : 128 + (c + 1) * 128] = Cpoly[0, c]
    # coefficient lookup tables for the per-element planes (class 0 used for
    # adj==0 elements; they are masked to NEG_INF anyway), k = 0..DEG
    clut = np.empty((DEG + 1, 6), np.float32)
    for k in range(DEG + 1):
        clut[k, 0] = Cpoly[k, 0]
        clut[k, 1:] = Cpoly[k, :]

    in_maps = []
    for core in range(NCORES):
        bs = slice(core * BL, (core + 1) * BL)
        hs = hidden[bs]                                   # (BL,N,D)
        # hT: [dl, (ch, b, n)]
        hT_host = np.ascontiguousarray(
            hs.reshape(BL, N, DCH, 128).transpose(3, 2, 0, 1)
        ).reshape(128, DCH * BL * N).astype(np.float16)
        # transposed score-space tensors: [j, (b, i)]
        A_host = np.ascontiguousarray(A[bs].transpose(2, 0, 1)).reshape(N, BL * N)
        adjT = adj[bs].transpose(2, 0, 1)                 # (j, b, i)
        assert ((adj[bs] >= 1) & (adj[bs] <= 5)).any(axis=2).all(), (
            "row with no valid edge: shift-free softmax unsupported")
        # int16-quantized coefficient planes (per-degree scale)
        cs = np.abs(Cpoly).max(axis=1) / 32767.0
        qlut = np.round(clut / cs[:, None].astype(np.float32)).astype(np.int16)
        fbC1_host = qlut[DEG, adjT].reshape(N, BL * N)
        fbC3_host = np.empty((N, 3 * BL * N), np.int16)
        for kk, deg in enumerate((3, 2, 1)):
            fbC3_host[:, kk * BL * N : (kk + 1) * BL * N] = (
                qlut[deg, adjT].reshape(N, BL * N))
        madj_host = np.empty((N, 5 * BL * N), np.int8)
        for r in range(5):                                # class r (adj==r+1)
            madj_host[:, r * BL * N : (r + 1) * BL * N] = (
                adjT == r + 1).reshape(N, BL * N)
        # h1: [j, (b, d|1)]
        h1_host = np.zeros((N, BL * (D + 4)), np.float16)
        for b in range(BL):
            h1_host[:, b * (D + 4) : b * (D + 4) + D] = hs[b]
            h1_host[:, b * (D + 4) + D] = 1.0
        in_maps.append({
            "hT": hT_host, "fbC1": fbC1_host, "fbC3": fbC3_host,
            "madj": madj_host, "h1": h1_host, "ap": ap_host,
            "A": A_host.astype(np.float16), "c0row": c0_host,
        })
    return nc, in_maps


# --------------------------------------------------------------------------
# public entry point
# --------------------------------------------------------------------------
def kernel(**inputs: np.ndarray) -> np.ndarray:
    nc, in_maps = _prepare(inputs)

    from concourse.bass_utils import run_bass_kernel_spmd

    res = run_bass_kernel_spmd(nc, in_maps, core_ids=list(range(NCORES)))
    out = np.empty((B, N, D), np.float32)
    for core in range(NCORES):
        o = res.results[core]["out"].astype(np.float32).reshape(N, BL, D)
        out[core * BL : (core + 1) * BL] = o.transpose(1, 0, 2)
    return out


if __name__ == "__main__":
    rng = np.random.default_rng(0)
    demo = {
        "hidden": rng.standard_normal((B, N, D), dtype=np.float32),
        "A_interval": rng.random((B, N, N), dtype=np.float32),
        "adj": rng.integers(0, 6, (B, N, N)).astype(np.int32),
        "interval_unique": rng.integers(0, 100, (B, N)).astype(np.int32),
        "mask_item": rng.integers(0, 2, (B, N)).astype(np.int32),
        "a_params": (rng.standard_normal((D, 5)) / np.sqrt(D)).astype(np.float32),
        "iw_params": rng.standard_normal((TDIM, 5)).astype(np.float32),
        "te_freq": rng.standard_normal(TDIM).astype(np.float32),
        "te_phase": rng.standard_normal(TDIM).astype(np.float32),
    }
    o = kernel(**demo)
    print("kernel output", o.shape, o.dtype, np.abs(o).max())


# revision 42
# speedup vs baseline: 1.0158x; 1.0158x over previous
"""Trainium2 Bass kernel for nn_LocalAggregator (GNN message passing).

Math (per batch):
    e[i,j,r] = lrelu( h_i . diag(a_r) . h_j  +  g_r(A_ij) )
    with g_r(a) = sum_t cos(a f_t + p_t) iw[t,r]
    s[i,j]   = e[i,j,adj_ij-1]  if 1<=adj<=5 else -9e15
    out      = softmax_j(s) @ h

Device strategy (per core, BL=4 of the 32 batches). Everything is computed
TRANSPOSED — scores live as s^T[j, (b,i)] — which removes the PE transpose
and yields the softmax denominator from an appended ones-column:
  * e1_c = H diag(a_c) H^T is symmetric, so psum[j, (c,i)-blocks] are fed
    by stationary hT_b (raw fp16) x moving hTa (a_c-scaled hT fp16, classes
    grouped in the moving free dim): per (b,ch) one 384-col + one 256-col
    fp16 matmul (1 cycle/row; plain fp32 is 4). One accumulation group per
    PSUM bank — interleaved open groups sharing a bank drop partial sums.
  * g_r(A): host fits a degree-4 polynomial per class AND RESOLVES THE
    CLASS SELECTION AT PACK TIME into per-element coefficient planes
    C_k[j,(b,i)] = c_k^(adj), shipped as int16 with a per-degree scale
    that folds into the STT for free:  t = (I_k * s_k) + t ; t = t * A.
    7 DVE ops total, no per-class work. The constant term c_0^(c) is
    accumulated into the e1 psum group by 1-row ones-stationary matmuls
    (emitted first, as the group start, in the PE-idle warmup window).
  * Class select for e1: Act broadcasts NEG_INF into s, then 5 DVE
    copy_predicated per b-pair half (int8 masks, straight from PSUM);
    poly fold + leaky relu are 2 more DVE stts per half (masked elements
    stay ~-9e15 -> exp == 0).
  * Softmax: exp(s - 8) on Act into fp16 (bias cancels in alpha; logit max
    is ~15.7 for this model so e^{s-8} fits fp16); denominator from the
    ones-column of the final fp16 matmul  alpha^T @ [h | 1 | pad]  (260
    cols; fp32r needs multiples of 4); per-partition 1/Z applied during
    the PSUM->SBUF copy on Act; output shipped fp16.
  * hTa build split DVE (fp16 tensor_scalar hits the 4x packed mode,
    ~345ns/op) / Act; madj+h1 ride the gpsimd software-DGE queue as a
    third DMA channel (safe only for late-consumed tensors).
  * Two walrus version-skew workarounds: the Tile tail drain and any
    instruction may carry at most ONE sync-wait command on this toolchain
    (_patch_tail_drain / _split_excess_waits hoist excess waits onto
    NoOps). Custom DVE ops and gpsimd wide tensor ops don't work on this
    toolchain/HW (codegen "ISA wrong length" / ~7.6us per 512-col op).
"""

from contextlib import ExitStack

import numpy as np

B, N, D, TDIM = 32, 128, 256, 64
NCORES = 8
BL = B // NCORES            # batches per core
ALPHA = 0.2
NEG_INF = -9e15
DEG = 4                     # host-fitted polynomial degree (fit err ~2e-3)
DEBUG_DUMP = False
DCH = D // 128              # K-chunks for the e1 contraction

_PROG_CACHE: dict = {}
_DRAIN_PATCHED = False


def _patch_tail_drain():
    """Version-skew workaround: the TileContext tail drain accumulates one
    sem-wait per outstanding engine/DMA queue, but this walrus build's Drain
    encoding fits only ONE sync-wait command. Spread the excess waits over
    preceding single-wait NoOps on the same (SP) engine."""
    global _DRAIN_PATCHED
    if _DRAIN_PATCHED:
        return
    import concourse.tile as tile_mod

    def _patched(self, tick_clock, wait_clock):
        nc = self.nc
        drain_inst = nc.sync.drain()
        wait_clock.add_sem_waits(
            drain_inst.ins,
            tile_mod.ScopedClock({None: tick_clock.global_clock}),
        )
        mi = drain_inst.ins
        si = mi.sync_info
        waits = list(si.on_wait) if si is not None and si.on_wait else []
        if len(waits) > 1:
            si.on_wait = waits[:1]
            lst = nc.cur_bb.bb.instructions
            assert lst[-1] is mi, "drain is not the last instruction in block"
            drain_obj = lst.pop()
            for w in waits[1:]:
                nop = nc.sync.nop(nofuse=True)
                nsi = nop.ins.sync_info
                if nsi is None:
                    nop.ins.sync_info = type(si)(on_update=[], on_wait=[w])
                else:
                    nsi.on_wait = [w]
            lst.append(drain_obj)
        nc.all_engine_barrier()
        assert self.sems is not None
        popped = nc._tile_sem_poison_stack.pop()
        assert popped is self._sem_poison
        nc.clear_and_free_semaphores(list(self.sems.allocated().values()))
        nc.all_engine_barrier()

    tile_mod.TileContext._drain_and_barrier = _patched
    _DRAIN_PATCHED = True


def _split_excess_waits(nc, max_waits: int = 1):
    """This walrus build encodes at most one sync-wait command per
    instruction. Hoist excess waits onto same-engine NoOps inserted
    immediately before the over-subscribed instruction."""
    import concourse.mybir as mybir

    for fn in nc.m.functions:
        for bb in fn.blocks:
            insts = bb.instructions
            i = 0
            while i < len(insts):
                inst = insts[i]
                si = getattr(inst, "sync_info", None)
                waits = list(si.on_wait) if si is not None and si.on_wait else []
                if len(waits) > max_waits:
                    si.on_wait = waits[:max_waits]
                    extra = waits[max_waits:]
                    nops = []
                    for k in range(0, len(extra), max_waits):
                        nops.append(
                            mybir.InstNoOp(
                                name=f"{inst.name}-xw{k}",
                                engine=inst.engine,
                                bass_nofuse=True,
                                sync_info=mybir.SyncInfo(
                                    on_wait=extra[k : k + max_waits], on_update=[]
                                ),
                            )
                        )
                    insts[i:i] = nops
                    i += len(nops)
                i += 1


# --------------------------------------------------------------------------
# host-side parameter preprocessing
# --------------------------------------------------------------------------
def _fit_polys(iw_params: np.ndarray, te_freq: np.ndarray, te_phase: np.ndarray):
    """Least-squares fit of g_c(a) = sum_t iw[t,c] cos(a f_t + p_t), a in [0,1].

    Returns C[k, c] for k=0..DEG (monomial basis, increasing order).
    """
    npts = 1024
    x = 0.5 * (1.0 + np.cos(np.pi * (np.arange(npts) + 0.5) / npts))
    f = te_freq.astype(np.float64)
    p = te_phase.astype(np.float64)
    iw = iw_params.astype(np.float64)
    G = np.cos(x[:, None] * f[None, :] + p[None, :]) @ iw      # (npts, 5)
    V = np.vander(x, DEG + 1, increasing=True)                 # (npts, DEG+1)
    C, *_ = np.linalg.lstsq(V, G, rcond=None)
    return C  # (DEG+1, 5) float64


# --------------------------------------------------------------------------
# Bass program
# --------------------------------------------------------------------------
def _build_program(Cpoly: np.ndarray):
    """One-core program; SPMD across 8 cores with per-core input maps."""
    CSCALE = np.abs(Cpoly).max(axis=1) / 32767.0
    import concourse.bass as bass
    import concourse.mybir as mybir
    import concourse.tile as tile

    _patch_tail_drain()

    f32 = mybir.dt.float32
    f32r = mybir.dt.float32r
    f16 = mybir.dt.float16
    i8 = mybir.dt.int8
    Alu = mybir.AluOpType
    Act = mybir.ActivationFunctionType

    nc = bass.Bass()

    FBJ = BL * N          # 512  free size of (b, x)
    FBD = BL * D          # 1024 free size of (b, d)
    HTC = DCH * FBJ       # 1024 hT cols (ch, b, n)

    # DRAM I/O (per-core layouts; host arranges)
    hT_d = nc.dram_tensor("hT", [128, HTC], f16, kind="ExternalInput")
    i16 = mybir.dt.int16
    fbC1_d = nc.dram_tensor("fbC1", [N, FBJ], i16, kind="ExternalInput")     # C4
    fbC3_d = nc.dram_tensor("fbC3", [N, 3 * FBJ], i16, kind="ExternalInput")  # C3 C2 C1
    c0_d = nc.dram_tensor("c0row", [1, 128 + 640], f32r, kind="ExternalInput")
    madj_d = nc.dram_tensor("madj", [N, 5 * FBJ], i8, kind="ExternalInput")
    h1_d = nc.dram_tensor("h1", [N, BL * (D + 4)], f16, kind="ExternalInput")
    ap_d = nc.dram_tensor("ap", [128, DCH * 5 + 2], f32, kind="ExternalInput")
    A_d = nc.dram_tensor("A", [N, FBJ], f16, kind="ExternalInput")
    out_d = nc.dram_tensor("out", [N, FBD], f16, kind="ExternalOutput")  # [i,(b,d)]
    if DEBUG_DUMP:
        dbgA_d = nc.dram_tensor("dbgA", [N, BL * 512], f32, kind="ExternalOutput")
        dbgB_d = nc.dram_tensor("dbgB", [N, BL * 512], f32, kind="ExternalOutput")
        dbgT_d = nc.dram_tensor("dbgT", [N, FBJ], f32, kind="ExternalOutput")
        dbgS_d = nc.dram_tensor("dbgS", [N, FBJ], f32, kind="ExternalOutput")

    with tile.TileContext(nc) as tc, ExitStack() as ctx:
        io = ctx.enter_context(tc.tile_pool(name="io", bufs=1))
        wrk = ctx.enter_context(tc.tile_pool(name="wrk", bufs=1))

        # ---- loads: sync queue feeds Act/PE chain, scalar queue feeds DVE
        s_sb = wrk.tile([N, FBJ], f32, tag="s")
        hT_sb = io.tile([128, HTC], f16, tag="hT")
        nc.sync.dma_start(hT_sb[:, 0:FBJ], hT_d[:, 0:FBJ])
        c0_sb = io.tile([1, 768], f32r, tag="c0row")
        nc.sync.dma_start(c0_sb[:], c0_d[:])
        fbC1_sb = io.tile([N, FBJ], i16, tag="fbC1")
        nc.sync.dma_start(fbC1_sb[:], fbC1_d[:])
        nc.sync.dma_start(hT_sb[:, FBJ : 2 * FBJ], hT_d[:, FBJ : 2 * FBJ])
        ap_sb_t = io.tile([128, DCH * 5 + 2], f32, tag="ap")
        nc.scalar.dma_start(ap_sb_t[:], ap_d[:])
        ap_sb = ap_sb_t[:]
        A16_sb = io.tile([N, FBJ], f16, tag="A16")
        nc.scalar.dma_start(A16_sb[:], A_d[:])
        A_sb_t = wrk.tile([N, FBJ], f32, tag="A")
        nc.scalar.copy(A_sb_t[:], A16_sb[:])
        A_sb = A_sb_t[:]
        fbC3_sb = io.tile([N, 3 * FBJ], i16, tag="fbC3")
        nc.scalar.dma_start(fbC3_sb[:], fbC3_d[:])
        madj_sb = io.tile([N, 5 * FBJ], i8, tag="madj")
        nc.gpsimd.dma_start(madj_sb[:], madj_d[:])
        h1_sb = io.tile([N, BL * (D + 4)], f16, tag="h1")
        nc.gpsimd.dma_start(h1_sb[:], h1_d[:])

        # ---- hTa[dl, (ch, c, b, n)] = a_c[dl] * hT  (Act for c<3, Pool c>=3)
        hTa = wrk.tile([128, DCH * 5 * FBJ], f16, tag="hTa")
        for ch in range(DCH):
            for c in range(5):
                dst = hTa[:, (ch * 5 + c) * FBJ : (ch * 5 + c + 1) * FBJ]
                hsrc = hT_sb[:, ch * FBJ : (ch + 1) * FBJ]
                scal = ap_sb[:, ch * 5 + c : ch * 5 + c + 1]
                if ch == 1:
                    nc.scalar.mul(dst, hsrc, scal)
                else:
                    nc.vector.tensor_scalar(dst, hsrc, scal, None, Alu.mult)
        hTa_v = hTa[:].rearrange("p (ch c b n) -> p ch c b n", ch=DCH, c=5, b=BL)

        # ---- DVE: t[j,(b,i)] = (((C4*A + C3)*A + C2)*A + C1)*A
        # (per-element class-selected coefficient planes; c0 comes via PE)
        t_sb = wrk.tile([N, FBJ], f32, tag="t")
        nc.vector.scalar_tensor_tensor(
            t_sb[:], fbC1_sb[:], float(CSCALE[4]), A_sb,
            Alu.mult, Alu.mult)
        for sk, cpl in [(float(CSCALE[3]), fbC3_sb[:, 0:FBJ]),
                        (float(CSCALE[2]), fbC3_sb[:, FBJ : 2 * FBJ]),
                        (float(CSCALE[1]), fbC3_sb[:, 2 * FBJ : 3 * FBJ])]:
            nc.vector.scalar_tensor_tensor(
                t_sb[:], cpl, sk, t_sb[:], Alu.mult, Alu.add)
            nc.vector.scalar_tensor_tensor(
                t_sb[:], t_sb[:], 1.0, A_sb, Alu.mult, Alu.mult)

        # ---- PE: e1 + c0 accumulation, fp32r
        # psA[j, b*512 + c*128 + i] c in 0..2 ; psB[j, b*256 + (c-3)*128 + i]
        psum_cm = tc.tile_pool(name="psum", bufs=1, space="PSUM")
        psum = psum_cm.__enter__()
        psA = psum.tile([N, BL * 512], f32, tag="psA")
        psB = psum.tile([N, BL * 512], f32, tag="psB")
        psA_v = psA[:].rearrange("p (b x) -> p b x", b=BL)
        psB_v = psB[:].rearrange("p (b x) -> p b x", b=BL)

        ones_st = c0_sb[:, 0:128]
        c0A = c0_sb[:, 128:512]          # (c,i) c in 0..2
        c0B = c0_sb[:, 512:768]          # (c,i) c in 3..4
        for b in range(BL):
            nc.tensor.matmul(psA[:, b * 512 : b * 512 + 384], ones_st, c0A,
                             start=True, stop=False)
            nc.tensor.matmul(psB[:, b * 512 : b * 512 + 256], ones_st, c0B,
                             start=True, stop=False)
        for b in range(BL):
            stat = hT_sb[:, b * N : (b + 1) * N]
            nc.tensor.matmul(psA[:, b * 512 : b * 512 + 384], stat,
                             hTa_v[:, 0, 0:3, b, :], start=False, stop=False)
            nc.tensor.matmul(psB[:, b * 512 : b * 512 + 256], stat,
                             hTa_v[:, 0, 3:5, b, :], start=False, stop=False)
        for b in range(BL):
            stat = hT_sb[:, FBJ + b * N : FBJ + (b + 1) * N]
            nc.tensor.matmul(psB[:, b * 512 : b * 512 + 256], stat,
                             hTa_v[:, 1, 3:5, b, :], start=False, stop=True)
            nc.tensor.matmul(psA[:, b * 512 : b * 512 + 384], stat,
                             hTa_v[:, 1, 0:3, b, :], start=False, stop=True)

        if DEBUG_DUMP:
            dbgA_sb = wrk.tile([N, BL * 512], f32, tag="dbgA")
            nc.vector.tensor_copy(dbgA_sb[:], psA[:])
            nc.sync.dma_start(dbgA_d[:], dbgA_sb[:])
            dbgB_sb = wrk.tile([N, BL * 512], f32, tag="dbgB")
            nc.vector.tensor_copy(dbgB_sb[:], psB[:])
            nc.sync.dma_start(dbgB_d[:], dbgB_sb[:])

        # ---- select: s = NEG_INF ; copy_predicated class planes from PSUM
        madj_v = madj_sb[:].rearrange("p (m b i) -> p m b i", m=5, b=BL)
        s_v = s_sb[:].rearrange("p (b i) -> p b i", b=BL)
        neg_bc = ap_sb[:, DCH * 5 : DCH * 5 + 1].broadcast_to((N, FBJ))
        nc.scalar.activation(s_sb[:], neg_bc, Act.Copy)
        for half in range(2):
            bh = slice(half * 2, half * 2 + 2)
            for r in range(5):
                src = (psA_v[:, bh, r * 128 : (r + 1) * 128] if r < 3
                       else psB_v[:, bh, (r - 3) * 128 : (r - 2) * 128])
                nc.vector.copy_predicated(s_v[:, bh], madj_v[:, r, bh], src)
        psum_cm.__exit__(None, None, None)

        if DEBUG_DUMP:
            nc.sync.dma_start(dbgT_d[:], t_sb[:])
            nc.sync.dma_start(dbgS_d[:], s_sb[:])

        # ---- Act: exp ; PE: alpha^T @ [h|1] ; Act: 1/Z scale ; DMA out
        ex = wrk.tile([N, FBJ], f16, tag="ex")
        rz = wrk.tile([N, BL], f32, tag="rz")
        out_sb = wrk.tile([N, FBD], f16, tag="out_sb")
        psum2 = ctx.enter_context(tc.tile_pool(name="psum2", bufs=2, space="PSUM"))
        for half in range(2):
            hs_ = slice(half * 256, (half + 1) * 256)
            # fold poly branch + leaky relu (per half, overlaps exp)
            nc.vector.scalar_tensor_tensor(
                s_sb[:, hs_], s_sb[:, hs_], 0.0, t_sb[:, hs_],
                Alu.add, Alu.add)
            nc.vector.scalar_tensor_tensor(
                s_sb[:, hs_], s_sb[:, hs_], ALPHA, s_sb[:, hs_],
                Alu.mult, Alu.max)
            for bb in (half * 2, half * 2 + 1):
                nc.scalar.activation(
                    ex[:, bb * N : (bb + 1) * N], s_sb[:, bb * N : (bb + 1) * N],
                    Act.Exp, bias=ap_sb[:, DCH * 5 + 1 : DCH * 5 + 2],
                )
        del half
        for b in range(BL):
            op2 = psum2.tile([N, D + 4], f32, tag="op2", name="op2")
            nc.tensor.matmul(
                op2[:],
                ex[:, b * N : (b + 1) * N],
                h1_sb[:, b * (D + 4) : (b + 1) * (D + 4)],
            )
            nc.vector.reciprocal(rz[:, b : b + 1], op2[:, D : D + 1])
            nc.scalar.mul(out_sb[:, b * D : (b + 1) * D], op2[:, 0:D],
                          rz[:, b : b + 1])
            nc.sync.dma_start(
                out_d[:, b * D : (b + 1) * D], out_sb[:, b * D : (b + 1) * D])

    return nc


# --------------------------------------------------------------------------
# host-side input prep (shared by kernel() and test.py's profiler)
# --------------------------------------------------------------------------
def _prepare(inputs):
    hidden = np.ascontiguousarray(inputs["hidden"], dtype=np.float32)   # (B,N,D)
    A = np.ascontiguousarray(inputs["A_interval"], dtype=np.float32)    # (B,N,N)
    adj = np.asarray(inputs["adj"])                                     # (B,N,N) i32
    a_params = np.asarray(inputs["a_params"], dtype=np.float32)         # (D,5)
    iw = np.asarray(inputs["iw_params"])
    f = np.asarray(inputs["te_freq"])
    p = np.asarray(inputs["te_phase"])

    Cpoly = _fit_polys(iw, f, p)

    key = Cpoly.tobytes()
    nc = _PROG_CACHE.get(key)
    if nc is None:
        nc = _build_program(Cpoly)
        _split_excess_waits(nc)
        _PROG_CACHE[key] = nc

    # a_params -> [dl, (ch, c)]
    ap_host = np.empty((128, DCH * 5 + 2), np.float32)
    for ch in range(DCH):
        ap_host[:, ch * 5 : (ch + 1) * 5] = a_params[ch * 128 : (ch + 1) * 128, :]
    ap_host[:, DCH * 5] = NEG_INF
    ap_host[:, DCH * 5 + 1] = -8.0
    c0_host = np.empty((1, 768), np.float32)
    c0_host[0, 0:128] = 1.0
    for c in range(5):
        c0_host[0, 128 + c * 128 : 128 + (c + 1) * 128] = Cpoly[0, c]
    # coefficient lookup tables for the per-element planes (class 0 used for
    # adj==0 elements; they are masked to NEG_INF anyway), k = 0..DEG
    clut = np.empty((DEG + 1, 6), np.float32)
    for k in range(DEG + 1):
        clut[k, 0] = Cpoly[k, 0]
        clut[k, 1:] = Cpoly[k, :]

    in_maps = []
    for core in range(NCORES):
        bs = slice(core * BL, (core + 1) * BL)
        hs = hidden[bs]                                   # (BL,N,D)
        # hT: [dl, (ch, b, n)]
        hT_host = np.ascontiguousarray(
            hs.reshape(BL, N, DCH, 128).transpose(3, 2, 0, 1)
        ).reshape(128, DCH * BL * N).astype(np.float16)
        # transposed score-space tensors: [j, (b, i)]
        A_host = np.ascontiguousarray(A[bs].transpose(2, 0, 1)).reshape(N, BL * N)
        adjT = adj[bs].transpose(2, 0, 1)                 # (j, b, i)
        assert ((adj[bs] >= 1) & (adj[bs] <= 5)).any(axis=2).all(), (
            "row with no valid edge: shift-free softmax unsupported")
        # int16-quantized coefficient planes (per-degree scale)
        cs = np.abs(Cpoly).max(axis=1) / 32767.0
        qlut = np.round(clut / cs[:, None].astype(np.float32)).astype(np.int16)
        fbC1_host = qlut[DEG, adjT].reshape(N, BL * N)
        fbC3_host = np.empty((N, 3 * BL * N), np.int16)
        for kk, deg in enumerate((3, 2, 1)):
            fbC3_host[:, kk * BL * N : (kk + 1) * BL * N] = (
                qlut[deg, adjT].reshape(N, BL * N))
        madj_host = np.empty((N, 5 * BL * N), np.int8)
        for r in range(5):                                # class r (adj==r+1)
            madj_host[:, r * BL * N : (r + 1) * BL * N] = (
                adjT == r + 1).reshape(N, BL * N)
        # h1: [j, (b, d|1)]
        h1_host = np.zeros((N, BL * (D + 4)), np.float16)
        for b in range(BL):
            h1_host[:, b * (D + 4) : b * (D + 4) + D] = hs[b]
            h1_host[:, b * (D + 4) + D] = 1.0
        in_maps.append({
            "hT": hT_host, "fbC1": fbC1_host, "fbC3": fbC3_host,
            "madj": madj_host, "h1": h1_host, "ap": ap_host,
            "A": A_host.astype(np.float16), "c0row": c0_host,
        })
    return nc, in_maps


# --------------------------------------------------------------------------
# public entry point
# --------------------------------------------------------------------------
def kernel(**inputs: np.ndarray) -> np.ndarray:
    nc, in_maps = _prepare(inputs)

    from concourse.bass_utils import run_bass_kernel_spmd

    res = run_bass_kernel_spmd(nc, in_maps, core_ids=list(range(NCORES)))
    out = np.empty((B, N, D), np.float32)
    for core in range(NCORES):
        o = res.results[core]["out"].astype(np.float32).reshape(N, BL, D)
        out[core * BL : (core + 1) * BL] = o.transpose(1, 0, 2)
    return out


if __name__ == "__main__":
    rng = np.random.default_rng(0)
    demo = {
        "hidden": rng.standard_normal((B, N, D), dtype=np.float32),
        "A_interval": rng.random((B, N, N), dtype=np.float32),
        "adj": rng.integers(0, 6, (B, N, N)).astype(np.int32),
        "interval_unique": rng.integers(0, 100, (B, N)).astype(np.int32),
        "mask_item": rng.integers(0, 2, (B, N)).astype(np.int32),
        "a_params": (rng.standard_normal((D, 5)) / np.sqrt(D)).astype(np.float32),
        "iw_params": rng.standard_normal((TDIM, 5)).astype(np.float32),
        "te_freq": rng.standard_normal(TDIM).astype(np.float32),
        "te_phase": rng.standard_normal(TDIM).astype(np.float32),
    }
    o = kernel(**demo)
    print("kernel output", o.shape, o.dtype, np.abs(o).max())


# revision 43
# speedup vs baseline: 1.0338x; 1.0178x over previous
"""Trainium2 Bass kernel for nn_LocalAggregator (GNN message passing).

Math (per batch):
    e[i,j,r] = lrelu( h_i . diag(a_r) . h_j  +  g_r(A_ij) )
    with g_r(a) = sum_t cos(a f_t + p_t) iw[t,r]
    s[i,j]   = e[i,j,adj_ij-1]  if 1<=adj<=5 else -9e15
    out      = softmax_j(s) @ h

Device strategy (per core, BL=4 of the 32 batches). Everything is computed
TRANSPOSED — scores live as s^T[j, (b,i)] — which removes the PE transpose
and yields the softmax denominator from an appended ones-column:
  * e1_c = H diag(a_c) H^T is symmetric, so psum[j, (c,i)-blocks] are fed
    by stationary hT_b (raw fp16) x moving hTa (a_c-scaled hT fp16, classes
    grouped in the moving free dim): per (b,ch) one 384-col + one 256-col
    fp16 matmul (1 cycle/row; plain fp32 is 4). One accumulation group per
    PSUM bank — interleaved open groups sharing a bank drop partial sums.
  * g_r(A): host fits a degree-4 polynomial per class AND RESOLVES THE
    CLASS SELECTION AT PACK TIME into per-element coefficient planes
    C_k[j,(b,i)] = c_k^(adj), shipped as int16 with a per-degree scale
    that folds into the STT for free:  t = (I_k * s_k) + t ; t = t * A.
    7 DVE ops total, no per-class work. The constant term c_0^(c) is
    accumulated into the e1 psum group by 1-row ones-stationary matmuls
    (emitted first, as the group start, in the PE-idle warmup window).
  * Class select for e1: Act broadcasts NEG_INF into s, then 5 DVE
    copy_predicated per b-pair half (int8 masks, straight from PSUM);
    poly fold + leaky relu are 2 more DVE stts per half (masked elements
    stay ~-9e15 -> exp == 0).
  * Softmax: exp(s - 8) on Act into fp16 (bias cancels in alpha; logit max
    is ~15.7 for this model so e^{s-8} fits fp16); denominator from the
    ones-column of the final fp16 matmul  alpha^T @ [h | 1 | pad]  (260
    cols; fp32r needs multiples of 4); per-partition 1/Z applied during
    the PSUM->SBUF copy on Act; output shipped fp16.
  * hTa build split DVE (fp16 tensor_scalar hits the 4x packed mode,
    ~345ns/op) / Act; madj+h1 ride the gpsimd software-DGE queue as a
    third DMA channel (safe only for late-consumed tensors).
  * Two walrus version-skew workarounds: the Tile tail drain and any
    instruction may carry at most ONE sync-wait command on this toolchain
    (_patch_tail_drain / _split_excess_waits hoist excess waits onto
    NoOps). Custom DVE ops and gpsimd wide tensor ops don't work on this
    toolchain/HW (codegen "ISA wrong length" / ~7.6us per 512-col op).
"""

from contextlib import ExitStack

import numpy as np

B, N, D, TDIM = 32, 128, 256, 64
NCORES = 8
BL = B // NCORES            # batches per core
ALPHA = 0.2
NEG_INF = -9e15
DEG = 4                     # host-fitted polynomial degree (fit err ~2e-3)
DEBUG_DUMP = False
DCH = D // 128              # K-chunks for the e1 contraction

_PROG_CACHE: dict = {}
_DRAIN_PATCHED = False


def _patch_tail_drain():
    """Version-skew workaround: the TileContext tail drain accumulates one
    sem-wait per outstanding engine/DMA queue, but this walrus build's Drain
    encoding fits only ONE sync-wait command. Spread the excess waits over
    preceding single-wait NoOps on the same (SP) engine."""
    global _DRAIN_PATCHED
    if _DRAIN_PATCHED:
        return
    import concourse.tile as tile_mod

    def _patched(self, tick_clock, wait_clock):
        nc = self.nc
        drain_inst = nc.sync.drain()
        wait_clock.add_sem_waits(
            drain_inst.ins,
            tile_mod.ScopedClock({None: tick_clock.global_clock}),
        )
        mi = drain_inst.ins
        si = mi.sync_info
        waits = list(si.on_wait) if si is not None and si.on_wait else []
        if len(waits) > 1:
            si.on_wait = waits[:1]
            lst = nc.cur_bb.bb.instructions
            assert lst[-1] is mi, "drain is not the last instruction in block"
            drain_obj = lst.pop()
            for w in waits[1:]:
                nop = nc.sync.nop(nofuse=True)
                nsi = nop.ins.sync_info
                if nsi is None:
                    nop.ins.sync_info = type(si)(on_update=[], on_wait=[w])
                else:
                    nsi.on_wait = [w]
            lst.append(drain_obj)
        nc.all_engine_barrier()
        assert self.sems is not None
        popped = nc._tile_sem_poison_stack.pop()
        assert popped is self._sem_poison
        nc.clear_and_free_semaphores(list(self.sems.allocated().values()))
        nc.all_engine_barrier()

    tile_mod.TileContext._drain_and_barrier = _patched
    _DRAIN_PATCHED = True


def _split_excess_waits(nc, max_waits: int = 1):
    """This walrus build encodes at most one sync-wait command per
    instruction. Hoist excess waits onto same-engine NoOps inserted
    immediately before the over-subscribed instruction."""
    import concourse.mybir as mybir

    for fn in nc.m.functions:
        for bb in fn.blocks:
            insts = bb.instructions
            i = 0
            while i < len(insts):
                inst = insts[i]
                si = getattr(inst, "sync_info", None)
                waits = list(si.on_wait) if si is not None and si.on_wait else []
                if len(waits) > max_waits:
                    si.on_wait = waits[:max_waits]
                    extra = waits[max_waits:]
                    nops = []
                    for k in range(0, len(extra), max_waits):
                        nops.append(
                            mybir.InstNoOp(
                                name=f"{inst.name}-xw{k}",
                                engine=inst.engine,
                                bass_nofuse=True,
                                sync_info=mybir.SyncInfo(
                                    on_wait=extra[k : k + max_waits], on_update=[]
                                ),
                            )
                        )
                    insts[i:i] = nops
                    i += len(nops)
                i += 1


# --------------------------------------------------------------------------
# host-side parameter preprocessing
# --------------------------------------------------------------------------
def _fit_polys(iw_params: np.ndarray, te_freq: np.ndarray, te_phase: np.ndarray):
    """Least-squares fit of g_c(a) = sum_t iw[t,c] cos(a f_t + p_t), a in [0,1].

    Returns C[k, c] for k=0..DEG (monomial basis, increasing order).
    """
    npts = 1024
    x = 0.5 * (1.0 + np.cos(np.pi * (np.arange(npts) + 0.5) / npts))
    f = te_freq.astype(np.float64)
    p = te_phase.astype(np.float64)
    iw = iw_params.astype(np.float64)
    G = np.cos(x[:, None] * f[None, :] + p[None, :]) @ iw      # (npts, 5)
    V = np.vander(x, DEG + 1, increasing=True)                 # (npts, DEG+1)
    C, *_ = np.linalg.lstsq(V, G, rcond=None)
    return C  # (DEG+1, 5) float64


# --------------------------------------------------------------------------
# Bass program
# --------------------------------------------------------------------------
def _build_program(Cpoly: np.ndarray):
    """One-core program; SPMD across 8 cores with per-core input maps."""
    CSCALE = np.abs(Cpoly).max(axis=1) / 32767.0
    import concourse.bass as bass
    import concourse.mybir as mybir
    import concourse.tile as tile

    _patch_tail_drain()

    f32 = mybir.dt.float32
    f32r = mybir.dt.float32r
    f16 = mybir.dt.float16
    i8 = mybir.dt.int8
    Alu = mybir.AluOpType
    Act = mybir.ActivationFunctionType

    nc = bass.Bass()

    FBJ = BL * N          # 512  free size of (b, x)
    FBD = BL * D          # 1024 free size of (b, d)
    HTC = DCH * FBJ       # 1024 hT cols (ch, b, n)

    # DRAM I/O (per-core layouts; host arranges)
    hT_d = nc.dram_tensor("hT", [128, HTC], f16, kind="ExternalInput")
    i16 = mybir.dt.int16
    fbC1_d = nc.dram_tensor("fbC1", [N, FBJ], i16, kind="ExternalInput")     # C4
    fbC3_d = nc.dram_tensor("fbC3", [N, 3 * FBJ], i16, kind="ExternalInput")  # C3 C2 C1
    c0_d = nc.dram_tensor("c0row", [1, 128 + 640], f32r, kind="ExternalInput")
    madj_d = nc.dram_tensor("madj", [N, 5 * FBJ], i8, kind="ExternalInput")
    h1_d = nc.dram_tensor("h1", [N, BL * (D + 4)], f16, kind="ExternalInput")
    ap_d = nc.dram_tensor("ap", [128, DCH * 5 + 2], f32, kind="ExternalInput")
    A_d = nc.dram_tensor("A", [N, FBJ], f16, kind="ExternalInput")
    out_d = nc.dram_tensor("out", [N, FBD], f16, kind="ExternalOutput")  # [i,(b,d)]
    if DEBUG_DUMP:
        dbgA_d = nc.dram_tensor("dbgA", [N, BL * 512], f32, kind="ExternalOutput")
        dbgB_d = nc.dram_tensor("dbgB", [N, BL * 512], f32, kind="ExternalOutput")
        dbgT_d = nc.dram_tensor("dbgT", [N, FBJ], f32, kind="ExternalOutput")
        dbgS_d = nc.dram_tensor("dbgS", [N, FBJ], f32, kind="ExternalOutput")

    with tile.TileContext(nc) as tc, ExitStack() as ctx:
        io = ctx.enter_context(tc.tile_pool(name="io", bufs=1))
        wrk = ctx.enter_context(tc.tile_pool(name="wrk", bufs=1))

        # ---- loads: sync queue feeds Act/PE chain, scalar queue feeds DVE
        s_sb = wrk.tile([N, FBJ], f32, tag="s")
        hT_sb = io.tile([128, HTC], f16, tag="hT")
        nc.sync.dma_start(hT_sb[:, 0:FBJ], hT_d[:, 0:FBJ])
        fbC1_sb = io.tile([N, FBJ], i16, tag="fbC1")
        nc.sync.dma_start(fbC1_sb[:], fbC1_d[:])
        c0_sb = io.tile([1, 768], f32r, tag="c0row")
        nc.sync.dma_start(c0_sb[:], c0_d[:])
        nc.sync.dma_start(hT_sb[:, FBJ : 2 * FBJ], hT_d[:, FBJ : 2 * FBJ])
        ap_sb_t = io.tile([128, DCH * 5 + 2], f32, tag="ap")
        nc.scalar.dma_start(ap_sb_t[:], ap_d[:])
        ap_sb = ap_sb_t[:]
        A16_sb = io.tile([N, FBJ], f16, tag="A16")
        nc.scalar.dma_start(A16_sb[:], A_d[:])
        A_sb_t = wrk.tile([N, FBJ], f32, tag="A")
        nc.vector.tensor_copy(A_sb_t[:], A16_sb[:])
        A_sb = A_sb_t[:]
        fbC3_sb = io.tile([N, 3 * FBJ], i16, tag="fbC3")
        nc.scalar.dma_start(fbC3_sb[:], fbC3_d[:])
        madj_sb = io.tile([N, 5 * FBJ], i8, tag="madj")
        nc.gpsimd.dma_start(madj_sb[:], madj_d[:])
        h1_sb = io.tile([N, BL * (D + 4)], f16, tag="h1")
        nc.gpsimd.dma_start(h1_sb[:], h1_d[:])

        # ---- hTa[dl, (ch, c, b, n)] = a_c[dl] * hT  (Act for c<3, Pool c>=3)
        hTa = wrk.tile([128, DCH * 5 * FBJ], f16, tag="hTa")
        for ch in range(DCH):
            for c in ((0, 1, 2, 3, 4) if ch == 0 else (3, 4, 0, 1, 2)):
                dst = hTa[:, (ch * 5 + c) * FBJ : (ch * 5 + c + 1) * FBJ]
                hsrc = hT_sb[:, ch * FBJ : (ch + 1) * FBJ]
                scal = ap_sb[:, ch * 5 + c : ch * 5 + c + 1]
                if ch == 1:
                    nc.scalar.mul(dst, hsrc, scal)
                else:
                    nc.vector.tensor_scalar(dst, hsrc, scal, None, Alu.mult)
        hTa_v = hTa[:].rearrange("p (ch c b n) -> p ch c b n", ch=DCH, c=5, b=BL)

        # ---- DVE: t[j,(b,i)] = (((C4*A + C3)*A + C2)*A + C1)*A
        # (per-element class-selected coefficient planes; c0 comes via PE)
        t_sb = wrk.tile([N, FBJ], f32, tag="t")
        nc.vector.scalar_tensor_tensor(
            t_sb[:], fbC1_sb[:], float(CSCALE[4]), A_sb,
            Alu.mult, Alu.mult)
        for sk, cpl in [(float(CSCALE[3]), fbC3_sb[:, 0:FBJ]),
                        (float(CSCALE[2]), fbC3_sb[:, FBJ : 2 * FBJ]),
                        (float(CSCALE[1]), fbC3_sb[:, 2 * FBJ : 3 * FBJ])]:
            nc.vector.scalar_tensor_tensor(
                t_sb[:], cpl, sk, t_sb[:], Alu.mult, Alu.add)
            nc.vector.scalar_tensor_tensor(
                t_sb[:], t_sb[:], 1.0, A_sb, Alu.mult, Alu.mult)

        # ---- PE: e1 + c0 accumulation, fp32r
        # psA[j, b*512 + c*128 + i] c in 0..2 ; psB[j, b*256 + (c-3)*128 + i]
        psum_cm = tc.tile_pool(name="psum", bufs=1, space="PSUM")
        psum = psum_cm.__enter__()
        psA = psum.tile([N, BL * 512], f32, tag="psA")
        psB = psum.tile([N, BL * 512], f32, tag="psB")
        psA_v = psA[:].rearrange("p (b x) -> p b x", b=BL)
        psB_v = psB[:].rearrange("p (b x) -> p b x", b=BL)

        ones_st = c0_sb[:, 0:128]
        c0A = c0_sb[:, 128:512]          # (c,i) c in 0..2
        c0B = c0_sb[:, 512:768]          # (c,i) c in 3..4
        for b in range(BL):
            nc.tensor.matmul(psA[:, b * 512 : b * 512 + 384], ones_st, c0A,
                             start=True, stop=False)
            nc.tensor.matmul(psB[:, b * 512 : b * 512 + 256], ones_st, c0B,
                             start=True, stop=False)
        for b in range(BL):
            stat = hT_sb[:, b * N : (b + 1) * N]
            nc.tensor.matmul(psA[:, b * 512 : b * 512 + 384], stat,
                             hTa_v[:, 0, 0:3, b, :], start=False, stop=False)
            nc.tensor.matmul(psB[:, b * 512 : b * 512 + 256], stat,
                             hTa_v[:, 0, 3:5, b, :], start=False, stop=False)
        for b in range(BL):
            stat = hT_sb[:, FBJ + b * N : FBJ + (b + 1) * N]
            nc.tensor.matmul(psB[:, b * 512 : b * 512 + 256], stat,
                             hTa_v[:, 1, 3:5, b, :], start=False, stop=True)
            nc.tensor.matmul(psA[:, b * 512 : b * 512 + 384], stat,
                             hTa_v[:, 1, 0:3, b, :], start=False, stop=True)

        if DEBUG_DUMP:
            dbgA_sb = wrk.tile([N, BL * 512], f32, tag="dbgA")
            nc.vector.tensor_copy(dbgA_sb[:], psA[:])
            nc.sync.dma_start(dbgA_d[:], dbgA_sb[:])
            dbgB_sb = wrk.tile([N, BL * 512], f32, tag="dbgB")
            nc.vector.tensor_copy(dbgB_sb[:], psB[:])
            nc.sync.dma_start(dbgB_d[:], dbgB_sb[:])

        # ---- select: s = NEG_INF ; copy_predicated class planes from PSUM
        madj_v = madj_sb[:].rearrange("p (m b i) -> p m b i", m=5, b=BL)
        s_v = s_sb[:].rearrange("p (b i) -> p b i", b=BL)
        neg_bc = ap_sb[:, DCH * 5 : DCH * 5 + 1].broadcast_to((N, FBJ))
        nc.scalar.activation(s_sb[:], neg_bc, Act.Copy)
        for half in range(2):
            bh = slice(half * 2, half * 2 + 2)
            for r in (3, 4, 0, 1, 2):
                src = (psA_v[:, bh, r * 128 : (r + 1) * 128] if r < 3
                       else psB_v[:, bh, (r - 3) * 128 : (r - 2) * 128])
                nc.vector.copy_predicated(s_v[:, bh], madj_v[:, r, bh], src)
        psum_cm.__exit__(None, None, None)

        if DEBUG_DUMP:
            nc.sync.dma_start(dbgT_d[:], t_sb[:])
            nc.sync.dma_start(dbgS_d[:], s_sb[:])

        # ---- Act: exp ; PE: alpha^T @ [h|1] ; Act: 1/Z scale ; DMA out
        ex = wrk.tile([N, FBJ], f16, tag="ex")
        rz = wrk.tile([N, BL], f32, tag="rz")
        out_sb = wrk.tile([N, FBD], f16, tag="out_sb")
        psum2 = ctx.enter_context(tc.tile_pool(name="psum2", bufs=2, space="PSUM"))
        for half in range(2):
            hs_ = slice(half * 256, (half + 1) * 256)
            # fold poly branch + leaky relu (per half, overlaps exp)
            nc.vector.scalar_tensor_tensor(
                s_sb[:, hs_], s_sb[:, hs_], 0.0, t_sb[:, hs_],
                Alu.add, Alu.add)
            nc.vector.scalar_tensor_tensor(
                s_sb[:, hs_], s_sb[:, hs_], ALPHA, s_sb[:, hs_],
                Alu.mult, Alu.max)
            for bb in (half * 2, half * 2 + 1):
                nc.scalar.activation(
                    ex[:, bb * N : (bb + 1) * N], s_sb[:, bb * N : (bb + 1) * N],
                    Act.Exp, bias=ap_sb[:, DCH * 5 + 1 : DCH * 5 + 2],
                )
        del half
        for b in range(BL):
            op2 = psum2.tile([N, D + 4], f32, tag="op2", name="op2")
            nc.tensor.matmul(
                op2[:],
                ex[:, b * N : (b + 1) * N],
                h1_sb[:, b * (D + 4) : (b + 1) * (D + 4)],
            )
            nc.vector.reciprocal(rz[:, b : b + 1], op2[:, D : D + 1])
            nc.scalar.mul(out_sb[:, b * D : (b + 1) * D], op2[:, 0:D],
                          rz[:, b : b + 1])
            nc.sync.dma_start(
                out_d[:, b * D : (b + 1) * D], out_sb[:, b * D : (b + 1) * D])

    return nc


# --------------------------------------------------------------------------
# host-side input prep (shared by kernel() and test.py's profiler)
# --------------------------------------------------------------------------
def _prepare(inputs):
    hidden = np.ascontiguousarray(inputs["hidden"], dtype=np.float32)   # (B,N,D)
    A = np.ascontiguousarray(inputs["A_interval"], dtype=np.float32)    # (B,N,N)
    adj = np.asarray(inputs["adj"])                                     # (B,N,N) i32
    a_params = np.asarray(inputs["a_params"], dtype=np.float32)         # (D,5)
    iw = np.asarray(inputs["iw_params"])
    f = np.asarray(inputs["te_freq"])
    p = np.asarray(inputs["te_phase"])

    Cpoly = _fit_polys(iw, f, p)

    key = Cpoly.tobytes()
    nc = _PROG_CACHE.get(key)
    if nc is None:
        nc = _build_program(Cpoly)
        _split_excess_waits(nc)
        _PROG_CACHE[key] = nc

    # a_params -> [dl, (ch, c)]
    ap_host = np.empty((128, DCH * 5 + 2), np.float32)
    for ch in range(DCH):
        ap_host[:, ch * 5 : (ch + 1) * 5] = a_params[ch * 128 : (ch + 1) * 128, :]
    ap_host[:, DCH * 5] = NEG_INF
    ap_host[:, DCH * 5 + 1] = -8.0
    c0_host = np.empty((1, 768), np.float32)
    c0_host[0, 0:128] = 1.0
    for c in range(5):
        c0_host[0, 128 + c * 128 : 128 + (c + 1) * 128] = Cpoly[0, c]
    # coefficient lookup tables for the per-element planes (class 0 used for
    # adj==0 elements; they are masked to NEG_INF anyway), k = 0..DEG
    clut = np.empty((DEG + 1, 6), np.float32)
    for k in range(DEG + 1):
        clut[k, 0] = Cpoly[k, 0]
        clut[k, 1:] = Cpoly[k, :]

    in_maps = []
    for core in range(NCORES):
        bs = slice(core * BL, (core + 1) * BL)
        hs = hidden[bs]                                   # (BL,N,D)
        # hT: [dl, (ch, b, n)]
        hT_host = np.ascontiguousarray(
            hs.reshape(BL, N, DCH, 128).transpose(3, 2, 0, 1)
        ).reshape(128, DCH * BL * N).astype(np.float16)
        # transposed score-space tensors: [j, (b, i)]
        A_host = np.ascontiguousarray(A[bs].transpose(2, 0, 1)).reshape(N, BL * N)
        adjT = adj[bs].transpose(2, 0, 1)                 # (j, b, i)
        assert ((adj[bs] >= 1) & (adj[bs] <= 5)).any(axis=2).all(), (
            "row with no valid edge: shift-free softmax unsupported")
        # int16-quantized coefficient planes (per-degree scale)
        cs = np.abs(Cpoly).max(axis=1) / 32767.0
        qlut = np.round(clut / cs[:, None].astype(np.float32)).astype(np.int16)
        fbC1_host = qlut[DEG, adjT].reshape(N, BL * N)
        fbC3_host = np.empty((N, 3 * BL * N), np.int16)
        for kk, deg in enumerate((3, 2, 1)):
            fbC3_host[:, kk * BL * N : (kk + 1) * BL * N] = (
                qlut[deg, adjT].reshape(N, BL * N))
        madj_host = np.empty((N, 5 * BL * N), np.int8)
        for r in range(5):                                # class r (adj==r+1)
            madj_host[:, r * BL * N : (r + 1) * BL * N] = (
                adjT == r + 1).reshape(N, BL * N)
        # h1: [j, (b, d|1)]
        h1_host = np.zeros((N, BL * (D + 4)), np.float16)
        for b in range(BL):
            h1_host[:, b * (D + 4) : b * (D + 4) + D] = hs[b]
            h1_host[:, b * (D + 4) + D] = 1.0
        in_maps.append({
            "hT": hT_host, "fbC1": fbC1_host, "fbC3": fbC3_host,
            "madj": madj_host, "h1": h1_host, "ap": ap_host,
            "A": A_host.astype(np.float16), "c0row": c0_host,
        })
    return nc, in_maps


# --------------------------------------------------------------------------
# public entry point
# --------------------------------------------------------------------------
def kernel(**inputs: np.ndarray) -> np.ndarray:
    nc, in_maps = _prepare(inputs)

    from concourse.bass_utils import run_bass_kernel_spmd

    res = run_bass_kernel_spmd(nc, in_maps, core_ids=list(range(NCORES)))
    out = np.empty((B, N, D), np.float32)
    for core in range(NCORES):
        o = res.results[core]["out"].astype(np.float32).reshape(N, BL, D)
        out[core * BL : (core + 1) * BL] = o.transpose(1, 0, 2)
    return out


if __name__ == "__main__":
    rng = np.random.default_rng(0)
    demo = {
        "hidden": rng.standard_normal((B, N, D), dtype=np.float32),
        "A_interval": rng.random((B, N, N), dtype=np.float32),
        "adj": rng.integers(0, 6, (B, N, N)).astype(np.int32),
        "interval_unique": rng.integers(0, 100, (B, N)).astype(np.int32),
        "mask_item": rng.integers(0, 2, (B, N)).astype(np.int32),
        "a_params": (rng.standard_normal((D, 5)) / np.sqrt(D)).astype(np.float32),
        "iw_params": rng.standard_normal((TDIM, 5)).astype(np.float32),
        "te_freq": rng.standard_normal(TDIM).astype(np.float32),
        "te_phase": rng.standard_normal(TDIM).astype(np.float32),
    }
    o = kernel(**demo)
    print("kernel output", o.shape, o.dtype, np.abs(o).max())
